# revision 2
# baseline (speedup 1.0000x reference)
"""GCNHead Trainium2 kernel v3 (8-core SPMD).

Math (matches reference):
  deg = bincount(dst)+1 (self loops);  dinv = deg^-1/2
  agg[n] = sum_{e: dst=n} dinv[src] * x[src]  + dinv[n] * x[n]
  h = (dinv[n] * agg[n]) @ W_gcn + b_gcn
  out = leaky_relu(h, 0.2);  pooled = segment_max(out, batch)
  result = pooled @ (W_fc / sigma_max(W_fc)).T + b_fc

v3 design (informed by HW gather microbenchmarks: 256B random gathers run
near HBM roofline on 4 SWDGE queues; the kernel is engine-balance-bound,
not descriptor-bound):
  - no phase 1: gathers read raw bf16 x pairs; dinv[src] folds into the
    one-hot values (oh = is_equal * rsqrt(deg_src)).
  - self loops never gathered: added as dinv^2 * x_slot (x permuted into
    slot order on host; ints-only host work).
  - parity classes per tile (64-wide one-hot, single is_equal pass).
  - one-hot layout [128e, 64s, blk] with a materialized iota-repeat table
    so every DVE operand has a packed innermost dim (2x DVE mode).
  - per-class edge buckets sorted by src; 2 gather calls per 8-tile batch.
"""
import sys

sys.path.insert(0, "/opt/trn_rl_repo")

import math
import os
import numpy as np
import ml_dtypes

import concourse.bass as bass
import concourse.mybir as mybir
import concourse.tile as tile
from concourse import bacc
from concourse.masks import make_identity

BF16 = mybir.dt.bfloat16
F32 = mybir.dt.float32
I16 = mybir.dt.int16

NCORES = 8
SENT = 20000.0  # slot sentinel (never matches iota 0..63)


# ----------------------------------------------------------------------------
# Host preprocessing (integers only)
# ----------------------------------------------------------------------------
def _preprocess(x, edge_index, batch, num_graphs):
    N, D = x.shape
    B = int(num_graphs)
    src = np.asarray(edge_index[0], dtype=np.int64)
    dst = np.asarray(edge_index[1], dtype=np.int64)
    batch = np.asarray(batch, dtype=np.int64)

    deg = np.bincount(dst, minlength=N).astype(np.int64) + 1  # + self loop

    counts_g = np.bincount(batch, minlength=B)
    starts_g = np.concatenate([[0], np.cumsum(counts_g)])

    GPC = math.ceil(B / NCORES)
    gw = np.add.reduceat(deg, starts_g[:-1]) if N else counts_g
    gw = np.where(counts_g > 0, gw, 0)
    order = np.argsort(-gw, kind="stable")
    core_graphs = [[] for _ in range(NCORES)]
    loads = np.zeros(NCORES)
    for g in order:
        c = int(np.argmin([loads[i] + (1e18 if len(core_graphs[i]) >= GPC else 0)
                           for i in range(NCORES)]))
        core_graphs[c].append(int(g))
        loads[c] += gw[g]
    for c in range(NCORES):
        core_graphs[c] += [-1] * (GPC - len(core_graphs[c]))
        # rank-sort within core (descending weight) so tile positions align
        # heavy-with-heavy across cores -> lower cross-core per-class max
        core_graphs[c].sort(key=lambda g: -(gw[g] if g >= 0 else -1))

    GCAP = 64 * max(1, math.ceil(counts_g.max() / 64))
    TPG = GCAP // 64
    TT = GPC * TPG
    S = TT * 64

    # --- slot assignment: per graph, balance per-parity edge counts across
    #     TPG bins (vector LPT) so per-(tile,parity) class maxima shrink ---
    ev_src = (src & 1) == 0
    deg_e = np.bincount(dst[ev_src], minlength=N)        # even-parity in-deg
    deg_o = np.bincount(dst[~ev_src], minlength=N)       # odd-parity in-deg
    node_slot = np.full(N, -1, dtype=np.int64)
    node_core = np.full(N, -1, dtype=np.int64)
    deg_slot = np.ones((NCORES, S), dtype=np.int64)
    slot_node = np.full((NCORES, S), -1, dtype=np.int64)
    BIG = np.iinfo(np.int64).max
    for c in range(NCORES):
        for gi, g in enumerate(core_graphs[c]):
            if g < 0:
                continue
            nodes = np.arange(starts_g[g], starts_g[g + 1])
            if len(nodes) == 0:
                continue
            nd = deg[nodes]
            nde = deg_e[nodes]
            ndo = deg_o[nodes]
            ordn = np.argsort(-nd, kind="stable")
            binload_e = np.zeros(TPG, dtype=np.int64)
            binload_o = np.zeros(TPG, dtype=np.int64)
            binfill = np.zeros(TPG, dtype=np.int64)
            for i in ordn:
                cost = np.maximum(binload_e + nde[i], binload_o + ndo[i])
                cost = np.where(binfill < 64, cost * TPG + binfill, BIG)
                b = int(np.argmin(cost))
                slot = gi * GCAP + b * 64 + binfill[b]
                node_slot[nodes[i]] = slot
                node_core[nodes[i]] = c
                deg_slot[c, slot] = nd[i]
                slot_node[c, slot] = nodes[i]
                binfill[b] += 1
                binload_e[b] += nde[i]
                binload_o[b] += ndo[i]

    # --- edges (NO self loops), bucketed per (core, tile, parity),
    #     sorted by src within bucket ---
    ecore = node_core[dst]
    eslot = node_slot[dst]
    etile = eslot >> 6
    edl = (eslot & 63).astype(np.int64)
    epar = (src & 1).astype(np.int64)
    episrc = src >> 1

    counts = np.zeros((NCORES, TT, 2), dtype=np.int64)
    per_core_order = []
    for c in range(NCORES):
        sel = np.where(ecore == c)[0]
        k = etile[sel] * 2 + epar[sel]
        o = np.lexsort((src[sel], k))
        sel = sel[o]
        per_core_order.append(sel)
        cnt = np.bincount(k[o], minlength=TT * 2)
        counts[c] = cnt.reshape(TT, 2)

    kmax = counts.max(axis=0)                       # [TT, 2]
    cap = ((kmax + 127) // 128) * 128
    blocks = cap // 128                             # [TT, 2]
    TOTBLK = int(blocks.sum())
    TOTPOS = TOTBLK * 128

    class_off = np.zeros((TT, 2), dtype=np.int64)
    pos = 0
    for t in range(TT):
        for q in range(2):
            class_off[t, q] = pos
            pos += cap[t, q]

    idx_tab = np.zeros((NCORES, TOTPOS), dtype=np.int64)
    dstl_tab = np.full((NCORES, TOTPOS), SENT, dtype=np.float32)
    degsrc_tab = np.ones((NCORES, TOTPOS), dtype=np.float32)
    for c in range(NCORES):
        sel = per_core_order[c]
        k = etile[sel] * 2 + epar[sel]
        cstart = np.concatenate([[0],
                                 np.cumsum(np.bincount(k, minlength=TT * 2))])
        within = np.arange(len(sel)) - cstart[k]
        gpos = class_off.reshape(-1)[k] + within
        idx_tab[c, gpos] = episrc[sel]
        dstl_tab[c, gpos] = edl[sel]
        degsrc_tab[c, gpos] = deg[src[sel]]

    idx16 = idx_tab.astype(np.int16).reshape(NCORES, TOTPOS // 16, 16)
    idx16 = np.ascontiguousarray(idx16.transpose(0, 2, 1))
    idx128 = np.tile(idx16, (1, 8, 1))
    dstl128 = np.ascontiguousarray(
        dstl_tab.reshape(NCORES, TOTBLK, 128).transpose(0, 2, 1)
    ).astype(ml_dtypes.bfloat16)
    degsrc128 = np.ascontiguousarray(
        degsrc_tab.reshape(NCORES, TOTBLK, 128).transpose(0, 2, 1)
    ).astype(ml_dtypes.bfloat16)

    # f-major slot tables [64, TT*64]: degs_rep[f, t*64+s] = deg_slot[t,s]
    # (replicated along f), xslotT[f, t*64+s] = x[slot_node[t,s], f]
    degs_rep = np.ones((NCORES, 64, S), dtype=ml_dtypes.bfloat16)
    xslotT = np.zeros((NCORES, 64, S), dtype=ml_dtypes.bfloat16)
    xf = np.asarray(x, dtype=np.float32)
    for c in range(NCORES):
        degs_rep[c] = np.broadcast_to(
            deg_slot[c].astype(ml_dtypes.bfloat16)[None, :], (64, S))
        sn = slot_node[c]                            # [S]
        valid = sn >= 0
        xs = np.zeros((S, D), dtype=np.float32)
        xs[valid] = xf[sn[valid]]
        xslotT[c] = np.ascontiguousarray(xs.T).astype(ml_dtypes.bfloat16)

    NP = ((N + 255) // 256) * 256
    x_pad = np.zeros((NP, D), dtype=ml_dtypes.bfloat16)
    x_pad[:N] = xf.astype(ml_dtypes.bfloat16)

    # constant iota-repeat [128, 64, GMAX]: value = s (built host-side; the
    # 2D-pattern device iota is not exercised on HW)
    TB = int(os.environ.get("GNN_TB", "8"))
    tile_caps = cap.sum(axis=1) // 128               # blocks per tile
    gmax = 0
    pos = 0
    tb0 = [0]
    for t in range(TT):
        pos += int(tile_caps[t])
        tb0.append(pos)
    for b0 in range(0, TT, TB):
        nb = min(TB, TT - b0)
        gmax = max(gmax, tb0[b0 + nb] - tb0[b0])
    iotarep = np.broadcast_to(
        np.arange(64, dtype=np.float32)[None, :, None], (128, 64, gmax)
    ).astype(ml_dtypes.bfloat16)
    iotarep = np.ascontiguousarray(iotarep)

    dims = dict(N=N, D=D, B=B, GPC=GPC, GCAP=GCAP, TPG=TPG, TT=TT, S=S, NP=NP,
                TOTBLK=TOTBLK, TOTPOS=TOTPOS,
                blocks=tuple(map(tuple, blocks)),
                kmax=tuple(map(tuple, kmax)))
    tables = dict(idx=idx128, dstl=dstl128, degsrc=degsrc128,
                  degs_rep=degs_rep, xslotT=xslotT, x_pad=x_pad,
                  iotarep=iotarep, core_graphs=core_graphs)
    return dims, tables


# ----------------------------------------------------------------------------
# Device program
# ----------------------------------------------------------------------------
def _build_program(dims):
    D = dims["D"]
    TT, TPG, GPC, GCAP = dims["TT"], dims["TPG"], dims["GPC"], dims["GCAP"]
    NP, TOTBLK, TOTPOS = dims["NP"], dims["TOTBLK"], dims["TOTPOS"]
    blocks = dims["blocks"]
    kmax = dims["kmax"]
    S = dims["S"]

    NQ = int(os.environ.get("GNN_QUEUES", "4"))
    CPB = int(os.environ.get("GNN_CALLS_PER_BATCH", "2"))
    nc = bacc.Bacc("TRN2", target_bir_lowering=False, debug=False,
                   num_swdge_queues=NQ)
    x_d = nc.dram_tensor("x", [NP, D], BF16, kind="ExternalInput")
    idx_d = nc.dram_tensor("idx", [128, TOTPOS // 16], I16,
                           kind="ExternalInput")
    dstl_d = nc.dram_tensor("dstl", [128, TOTBLK], BF16, kind="ExternalInput")
    degsrc_d = nc.dram_tensor("degsrc", [128, TOTBLK], BF16,
                              kind="ExternalInput")
    degsrep_d = nc.dram_tensor("degs_rep", [64, S], BF16, kind="ExternalInput")
    xslotT_d = nc.dram_tensor("xslotT", [64, S], BF16, kind="ExternalInput")
    wgcn_d = nc.dram_tensor("wgcn", [D, D], BF16, kind="ExternalInput")
    bgcn_d = nc.dram_tensor("bgcn", [D, 1], F32, kind="ExternalInput")
    wfc_d = nc.dram_tensor("wfc", [D, D], F32, kind="ExternalInput")
    bfc_d = nc.dram_tensor("bfc", [D, 1], F32, kind="ExternalInput")
    out_d = nc.dram_tensor("out", [D, GPC], F32, kind="ExternalOutput")

    # per-tile block layout (even class blocks then odd, per tile)
    tile_blk0 = []
    pos = 0
    for t in range(TT):
        tile_blk0.append((pos, blocks[t][0], blocks[t][1]))
        pos += blocks[t][0] + blocks[t][1]
    tile_blk0.append((pos, 0, 0))
    TB = int(os.environ.get("GNN_TB", "8"))   # tiles per batch
    GMAX = 0
    for b0 in range(0, TT, TB):
        nb = min(TB, TT - b0)
        GMAX = max(GMAX, tile_blk0[b0 + nb][0] - tile_blk0[b0][0])

    iotarep_d = nc.dram_tensor("iotarep", [128, 64 * GMAX], BF16,
                               kind="ExternalInput")
    x_pairs = x_d[:].rearrange("(v two) d -> v (two d)", two=2)

    with tile.TileContext(nc) as tc:
        with tc.tile_pool(name="consts", bufs=1) as cp:
            iotarep_t = cp.tile([128, 64, GMAX], BF16)
            nc.sync.dma_start(
                out=iotarep_t[:].rearrange("p a b -> p (a b)"),
                in_=iotarep_d[:])
            dstl_t = cp.tile([128, TOTBLK], BF16)
            nc.sync.dma_start(out=dstl_t[:], in_=dstl_d[:])
            idx_t = cp.tile([128, TOTPOS // 16], I16)
            nc.sync.dma_start(out=idx_t[:], in_=idx_d[:])
            degsrc_t = cp.tile([128, TOTBLK], BF16)
            nc.sync.dma_start(out=degsrc_t[:], in_=degsrc_d[:])
            degsrep_t = cp.tile([64, S], BF16)
            xslotT_t = cp.tile([64, S], BF16)
            wgcn_t = cp.tile([D, D], BF16)
            nc.sync.dma_start(out=wgcn_t[:], in_=wgcn_d[:])
            bgcn_t = cp.tile([D, 1], F32)
            nc.sync.dma_start(out=bgcn_t[:], in_=bgcn_d[:])
            wfc_t = cp.tile([D, D], F32)
            nc.sync.dma_start(out=wfc_t[:], in_=wfc_d[:])
            bfc_t = cp.tile([D, 1], F32)
            nc.sync.dma_start(out=bfc_t[:], in_=bfc_d[:])

            ident_t = cp.tile([64, 64], F32)
            make_identity(nc, ident_t[:])

            # dinv_src per edge position [128, TOTBLK] bf16 (device float math)
            dinvsrc_t = cp.tile([128, TOTBLK], BF16)
            with nc.allow_low_precision(
                    reason="dinv_src is a bf16 edge weight; 2e-2 tolerance"):
                nc.vector.reciprocal(dinvsrc_t[:], degsrc_t[:])
                nc.scalar.activation(dinvsrc_t[:], dinvsrc_t[:],
                                     mybir.ActivationFunctionType.Sqrt)
            dinvsrep_t = cp.tile([64, S], BF16)
            selfaddT_t = cp.tile([64, S], BF16)
            rec_t = cp.tile([64, S], BF16)
            _setup_done = [False]

            def _emit_slot_setup():
                # emitted after batch 0's one-hot so these DVE ops don't
                # delay the first gathers/oh in the in-order DVE queue
                if _setup_done[0]:
                    return
                _setup_done[0] = True
                nc.sync.dma_start(out=degsrep_t[:], in_=degsrep_d[:])
                nc.sync.dma_start(out=xslotT_t[:], in_=xslotT_d[:])
                with nc.allow_low_precision(
                        reason="bf16 slot tables; 2e-2 tolerance"):
                    # dinvsrep[f,(t,s)] = deg^-1/2, selfadd = deg^-1 * x
                    nc.vector.reciprocal(rec_t[:], degsrep_t[:])
                    nc.scalar.activation(dinvsrep_t[:], rec_t[:],
                                         mybir.ActivationFunctionType.Sqrt)
                    nc.vector.tensor_tensor(out=selfaddT_t[:],
                                            in0=xslotT_t[:], in1=rec_t[:],
                                            op=mybir.AluOpType.mult)

            REPEAT = int(os.environ.get("GNN_REPEAT", "1"))
            STAGE = int(os.environ.get("GNN_STAGE", "9"))
            # queue rotation must track the global Pool-DMA ordinal (the tile
            # framework assigns DMASW sem lanes round-robin across ALL Pool
            # DMA instructions), so do NOT reset per repeat iteration
            _QRR = [0]
            for _it in range(REPEAT):
                pooledT = cp.tile([64, GPC], BF16, tag="pool")
                hT = cp.tile([64, S], BF16, tag="hT")

                NBUF = int(os.environ.get("GNN_BUFS", "2"))
                TBUF = int(os.environ.get("GNN_TBUFS", "2"))
                PABUF = int(os.environ.get("GNN_PABUFS", "2"))
                PMBUF = int(os.environ.get("GNN_PMBUFS", "2"))
                gp = tc.alloc_tile_pool(name=f"gath_{_it}", bufs=NBUF)
                ohp = tc.alloc_tile_pool(name=f"oh_{_it}", bufs=NBUF)
                tp = tc.alloc_tile_pool(name=f"tail_{_it}", bufs=TBUF)
                pa = tc.alloc_tile_pool(name=f"psum_acc_{_it}", bufs=PABUF,
                                        space="PSUM")
                pm = tc.alloc_tile_pool(name=f"psum_misc_{_it}", bufs=PMBUF,
                                        space="PSUM")
                _spec = []

                def _emit_spectral():
                    # ---------------- spectral norm sigma(W_fc) + FC ------------
                    pf = tc.alloc_tile_pool(name=f"psum_fc_{_it}", bufs=1,
                                            space="PSUM")
                    mp = pf.tile([D, D], F32, tag="mp")
                    nc.tensor.matmul(out=mp[:], lhsT=wfc_t[:], rhs=wfc_t[:],
                                     start=True, stop=True)
                    m1_sb = cp.tile([D, D], F32, tag="m1sb")
                    nc.scalar.copy(out=m1_sb[:], in_=mp[:])
                    cur = m1_sb
                    for _p in range(6):  # M^64
                        mp2 = pf.tile([D, D], F32, tag="mp")
                        nc.tensor.matmul(out=mp2[:], lhsT=cur[:], rhs=cur[:],
                                         start=True, stop=True)
                        nxt = cp.tile([D, D], F32, tag=f"m{_p}")
                        nc.scalar.copy(out=nxt[:], in_=mp2[:])
                        cur = nxt
                    ones_c = cp.tile([D, 1], F32, tag="oc")
                    nc.vector.memset(ones_c[:], 1.0)
                    ones_r = cp.tile([1, D], F32, tag="orr")
                    nc.vector.memset(ones_r[:], 1.0)
                    vp = pf.tile([D, 1], F32, tag="vp")
                    nc.tensor.matmul(out=vp[:], lhsT=cur[:], rhs=ones_c[:],
                                     start=True, stop=True)
                    v_sb = cp.tile([D, 1], F32, tag="vsb")
                    nc.scalar.copy(out=v_sb[:], in_=vp[:])
                    wp = pf.tile([D, 1], F32, tag="vp")
                    nc.tensor.matmul(out=wp[:], lhsT=m1_sb[:], rhs=v_sb[:],
                                     start=True, stop=True)
                    w_sb = cp.tile([D, 1], F32, tag="wsb")
                    nc.scalar.copy(out=w_sb[:], in_=wp[:])
                    nump = pf.tile([1, 1], F32, tag="sc")
                    nc.tensor.matmul(out=nump[:], lhsT=v_sb[:], rhs=w_sb[:],
                                     start=True, stop=True)
                    denp = pf.tile([1, 1], F32, tag="sc")
                    nc.tensor.matmul(out=denp[:], lhsT=v_sb[:], rhs=v_sb[:],
                                     start=True, stop=True)
                    num_sb = cp.tile([1, 1], F32, tag="num")
                    den_sb = cp.tile([1, 1], F32, tag="den")
                    nc.vector.tensor_copy(out=num_sb[:], in_=nump[:])
                    nc.vector.tensor_copy(out=den_sb[:], in_=denp[:])
                    rinv = cp.tile([1, 1], F32, tag="rinv")
                    nc.vector.reciprocal(rinv[:], num_sb[:])
                    nc.vector.tensor_tensor(out=rinv[:], in0=rinv[:],
                                            in1=den_sb[:],
                                            op=mybir.AluOpType.mult)
                    nc.scalar.activation(rinv[:], rinv[:],
                                         mybir.ActivationFunctionType.Sqrt)
                    sp = pf.tile([D, 1], F32, tag="vp")
                    nc.tensor.matmul(out=sp[:], lhsT=ones_r[:], rhs=rinv[:],
                                     start=True, stop=True)
                    s_col = cp.tile([D, 1], F32, tag="scol")
                    nc.scalar.copy(out=s_col[:], in_=sp[:])

                    wtp = pf.tile([D, D], F32, tag="mp")
                    nc.tensor.transpose(out=wtp[:], in_=wfc_t[:],
                                        identity=ident_t[:D, :D])
                    wfcT = cp.tile([D, D], BF16, tag="wfcT")
                    nc.vector.tensor_scalar_mul(wfcT[:], wtp[:], s_col[:])
                    return pf, wfcT


                for b0 in range(0, TT, TB):
                    nb = min(TB, TT - b0)
                    blk0 = tile_blk0[b0][0]
                    blk1 = tile_blk0[b0 + nb][0]
                    nblk = blk1 - blk0
                    gat = gp.tile([128, GMAX, 128], BF16, tag="gat")
                    oh = ohp.tile([128, 64, GMAX], BF16, tag="oh")
                    if nblk > 0:
                        # HW caps dma_gather somewhere <=2048 idxs/call
                        # (4096-idx calls die with NRT INTERNAL): use 8-block
                        # (1024-idx) calls like v1, rotating queues
                        per_c = 8
                        for g0 in range(0, nblk, per_c):
                            gl = min(per_c, nblk - g0)
                            nc.gpsimd.dma_gather(
                                out_ap=gat[:, g0:g0 + gl, :],
                                in_ap=x_pairs,
                                idxs_ap=idx_t[:, (blk0 + g0) * 8:
                                              (blk0 + g0 + gl) * 8],
                                num_idxs=gl * 128,
                                num_idxs_reg=gl * 128,
                                elem_size=128,
                                queue_num=_QRR[0],
                            )
                            _QRR[0] = (_QRR[0] + 1) % NQ
                        if STAGE < 3:
                            continue
                        # one-hot [128e, 64s, nblk]: is_equal then *dinv_src;
                        # all operands have packed innermost (2x DVE)
                        nc.vector.tensor_tensor(
                            out=oh[:, :, :nblk],
                            in0=iotarep_t[:, :, :nblk],
                            in1=dstl_t[:, blk0:blk1].rearrange(
                                "p (j b) -> p j b",
                                j=1).to_broadcast([128, 64, nblk]),
                            op=mybir.AluOpType.is_equal,
                        )
                        nc.vector.tensor_tensor(
                            out=oh[:, :, :nblk],
                            in0=oh[:, :, :nblk],
                            in1=dinvsrc_t[:, blk0:blk1].rearrange(
                                "p (j b) -> p j b",
                                j=1).to_broadcast([128, 64, nblk]),
                            op=mybir.AluOpType.mult,
                        )
                    # transposed accumulation: accT[f, s] (swap lhsT/rhs)
                    if STAGE < 4:
                        continue
                    _emit_slot_setup()
                    acc = pa.tile([64, TB, 64], F32, tag="acc")
                    for ti in range(nb):
                        t = b0 + ti
                        base, be, bo = tile_blk0[t]
                        nblks_t = be + bo
                        for j in range(nblks_t):
                            q = 0 if j < be else 1
                            jj = j if j < be else j - be
                            k = 128
                            if jj == (be if q == 0 else bo) - 1:
                                k = kmax[t][q] - 128 * jj
                                k = 128 if k <= 0 else k
                            lb = base - blk0 + j
                            nc.tensor.matmul(
                                out=acc[:, ti, :],
                                lhsT=gat[:k, lb, 64 * q:64 * q + 64],
                                rhs=oh[:k, :, lb],
                                start=(j == 0),
                                stop=(j == nblks_t - 1),
                            )
                        if nblks_t == 0:
                            nc.vector.memset(acc[:, ti, :], 0.0)

                    if STAGE < 5:
                        continue
                    # agg_sc[f,s] = dinv[s]*accT + dinv2[s]*x[s]  (self loop)
                    agg_sc = tp.tile([64, TB, 64], BF16, tag="aggsc")
                    nc.vector.tensor_tensor(
                        out=agg_sc[:, :nb, :], in0=acc[:, :nb, :],
                        in1=dinvsrep_t[:, b0 * 64:(b0 + nb) * 64].rearrange(
                            "p (a b) -> p a b", b=64),
                        op=mybir.AluOpType.mult,
                    )
                    nc.vector.tensor_tensor(
                        out=agg_sc[:, :nb, :], in0=agg_sc[:, :nb, :],
                        in1=selfaddT_t[:, b0 * 64:(b0 + nb) * 64].rearrange(
                            "p (a b) -> p a b", b=64),
                        op=mybir.AluOpType.add,
                    )
                    HB = 7 if TB >= 14 else TB      # tiles per psum bank
                    NCHUNK = (TB + HB - 1) // HB
                    # bank-aligned chunks: 512 f32 = one 2KB bank each
                    hps = pm.tile([64, NCHUNK, 512], F32, tag="hps")
                    for hi in range(NCHUNK):
                        h0 = hi * HB
                        hl = min(HB, nb - h0)
                        if hl <= 0:
                            break
                        nc.tensor.matmul(
                            out=hps[:, hi, :hl * 64],
                            lhsT=wgcn_t[:],
                            rhs=agg_sc[:, h0:h0 + hl, :].rearrange(
                                "p a b -> p (a b)"),
                            start=True, stop=True,
                        )
                        nc.scalar.activation(
                            out=hT[:, (b0 + h0) * 64:(b0 + h0 + hl) * 64],
                            in_=hps[:, hi, :hl * 64],
                            func=mybir.ActivationFunctionType.Identity,
                            bias=bgcn_t[:],
                        )
                    if b0 == TB:
                        _spec.extend(_emit_spectral())
                    lk = tp.tile([64, TB * 64], BF16, tag="lk")
                    nc.vector.tensor_scalar_mul(
                        lk[:, :nb * 64], hT[:, b0 * 64:(b0 + nb) * 64], 0.2)
                    nc.vector.tensor_tensor(
                        out=hT[:, b0 * 64:(b0 + nb) * 64],
                        in0=hT[:, b0 * 64:(b0 + nb) * 64],
                        in1=lk[:, :nb * 64], op=mybir.AluOpType.max)
                if STAGE < 6:
                    zo = cp.tile([D, GPC], F32, tag="zo")
                    nc.vector.memset(zo[:], 0.0)
                    nc.sync.dma_start(out=out_d[:], in_=zo[:])
                    for _pool in (pm, pa, tp, ohp, gp):
                        _pool.release()
                    continue
                for g in range(GPC):
                    nc.vector.tensor_reduce(
                        out=pooledT[:, g:g + 1],
                        in_=hT[:, g * GCAP:(g + 1) * GCAP],
                        axis=mybir.AxisListType.X,
                        op=mybir.AluOpType.max,
                    )

                pf, wfcT = _spec
                op_ = pf.tile([D, GPC], F32, tag="op")
                nc.tensor.matmul(out=op_[:], lhsT=wfcT[:], rhs=pooledT[:],
                                 start=True, stop=True)
                out_sb = cp.tile([D, GPC], F32, tag="osb")
                nc.scalar.activation(out=out_sb[:], in_=op_[:],
                                     func=mybir.ActivationFunctionType.Identity,
                                     bias=bfc_t[:])
                nc.sync.dma_start(out=out_d[:], in_=out_sb[:])
                pf.release()
                for _pool in (pm, pa, tp, ohp, gp):
                    _pool.release()

            _emit_slot_setup()

    nc.compile()
    return nc


# ----------------------------------------------------------------------------
# Cached executor (compile once per dims signature)
# ----------------------------------------------------------------------------
_CACHE = {}


class _Exec:
    def __init__(self, dims):
        self.dims = dims
        self.nc = _build_program(dims)

    def run(self, in_maps):
        from concourse.bass_utils import run_bass_kernel_spmd
        res = run_bass_kernel_spmd(self.nc, in_maps, list(range(NCORES)))
        return [r["out"] for r in res.results]


def _get_exec(dims):
    key = repr(sorted(dims.items()))
    if key not in _CACHE:
        _CACHE[key] = _Exec(dims)
    return _CACHE[key]


def _make_in_maps(dims, tables, W_gcn, b_gcn, W_fc, b_fc):
    wgcn = np.asarray(W_gcn, dtype=np.float32).astype(ml_dtypes.bfloat16)
    bgcn = np.asarray(b_gcn, dtype=np.float32).reshape(-1, 1)
    wfc = np.asarray(W_fc, dtype=np.float32)
    bfc = np.asarray(b_fc, dtype=np.float32).reshape(-1, 1)
    in_maps = []
    for c in range(NCORES):
        in_maps.append({
            "x": tables["x_pad"],
            "idx": np.ascontiguousarray(tables["idx"][c]),
            "dstl": np.ascontiguousarray(tables["dstl"][c]),
            "degsrc": np.ascontiguousarray(tables["degsrc"][c]),
            "degs_rep": np.ascontiguousarray(tables["degs_rep"][c]),
            "iotarep": tables["iotarep"].reshape(128, -1),
            "xslotT": np.ascontiguousarray(tables["xslotT"][c]),
            "wgcn": wgcn,
            "bgcn": bgcn,
            "wfc": wfc,
            "bfc": bfc,
        })
    return in_maps


def kernel(x, W_gcn, b_gcn, W_fc, b_fc, edge_index, batch, num_graphs):
    dims, tables = _preprocess(x, edge_index, batch, num_graphs)
    ex = _get_exec(dims)
    in_maps = _make_in_maps(dims, tables, W_gcn, b_gcn, W_fc, b_fc)
    outs = ex.run(in_maps)
    B = dims["B"]
    D = dims["D"]
    result = np.zeros((B, D), dtype=np.float32)
    for c in range(NCORES):
        o = np.asarray(outs[c], dtype=np.float32)  # [D, GPC]
        for gi, g in enumerate(tables["core_graphs"][c]):
            if g >= 0:
                result[g] = o[:, gi]
    return result


# ----------------------------------------------------------------------------
# Reusable jitted runner (for steady-state timing): mirrors
# bass2jax.run_bass_via_pjrt's multi-core path but keeps the jitted callable.
# ----------------------------------------------------------------------------
def _build_jit(nc):
    import jax
    import numpy as _np
    from jax.sharding import Mesh, PartitionSpec
    from jax.experimental.shard_map import shard_map
    from concourse import bass2jax
    from concourse import mybir as _mb

    bass2jax.install_neuronx_cc_hook()
    in_names, out_names, out_avals, zero_outs = [], [], [], []
    partition_name = (nc.partition_id_tensor.name
                      if nc.partition_id_tensor else None)
    for alloc in nc.m.functions[0].allocations:
        if not isinstance(alloc, _mb.MemoryLocationSet):
            continue
        name = alloc.memorylocations[0].name
        if alloc.kind == "ExternalInput":
            if name != partition_name:
                in_names.append(name)
        elif alloc.kind == "ExternalOutput":
            out_names.append(name)
            shape = tuple(alloc.tensor_shape)
            dtype = _mb.dt.np(alloc.dtype)
            out_avals.append(jax.core.ShapedArray(shape, dtype))
            zero_outs.append(_np.zeros(shape, dtype))
    n_params = len(in_names)
    all_in = list(in_names) + list(out_names)
    if partition_name is not None:
        all_in.append(partition_name)

    def _body(*args):
        operands = list(args)
        if partition_name is not None:
            operands.append(bass2jax.partition_id_tensor())
        outs = bass2jax._bass_exec_p.bind(
            *operands,
            out_avals=tuple(out_avals),
            in_names=tuple(all_in),
            out_names=tuple(out_names),
            lowering_input_output_aliases=(),
            sim_require_finite=True,
            sim_require_nnan=True,
            nc=nc,
        )
        return tuple(outs)

    devices = jax.devices()[:NCORES]
    mesh = Mesh(np.asarray(devices), ("core",))
    in_specs = (PartitionSpec("core"),) * (n_params + len(out_names))
    out_specs = (PartitionSpec("core"),) * len(out_names)
    donate = tuple(range(n_params, n_params + len(out_names)))
    fn = jax.jit(
        shard_map(_body, mesh=mesh, in_specs=in_specs, out_specs=out_specs,
                  check_rep=False),
        donate_argnums=donate, keep_unused=True,
    )
    return fn, in_names, out_names, zero_outs


def time_exec(ex, in_maps, reps=24, warmup=4):
    """Median per-iteration device time via back-to-back dispatch."""
    import jax
    import time as _t
    fn, in_names, out_names, zero_outs = _build_jit(ex.nc)
    concat = [np.concatenate([np.asarray(in_maps[c][n]) for c in range(NCORES)],
                             axis=0) for n in in_names]
    dev_in = [jax.device_put(a) for a in concat]
    for a in dev_in:
        a.block_until_ready()

    def zouts():
        return [np.concatenate([z] * NCORES, axis=0) for z in zero_outs]

    def run_n(n):
        outs = None
        t0 = _t.perf_counter()
        for _ in range(n):
            outs = fn(*dev_in, *zouts())
        for o in outs:
            o.block_until_ready()
        return _t.perf_counter() - t0

    run_n(warmup)
    t1 = run_n(reps // 2)
    t2 = run_n(reps)
    per_iter = (t2 - t1) / (reps - reps // 2)
    return per_iter * 1e9

